# revision 1
# baseline (speedup 1.0000x reference)
"""DSC layer (moe_routing) on 8 TRN2 NeuronCores, data-parallel over tokens.

Math (per token n):
  r0[nb]   = sum_d x[n,d]*g[d]*rW[nb,d]            (bf16 matmul)
  r_raw    = rs[n]*r0 - rs[n]*mu[n]*sg[nb] + c[nb] (LN folded into scalars)
  alpha    = softplus(clip(r_raw, +-10))
  top-8 of alpha via HW max8 + match_replace -> masked alpha (Zscat)
  q[n]     = tanh(S)/(S+eps), S = sum of top-8
  h_full   = x @ U_norm.T ; G = Zscat*q*h_full
  dyn      = G @ (V_norm * gamma)   (accumulated into the same PSUM as static)
  static   = gelu(x@W1.T) @ W2.T
All matmuls bf16 (fp32 accum). Stats (mean/var) computed in f32 via bn_stats.
All transposed layouts are prepared host-side (free); only math runs on device.
"""
import sys, os
sys.path.insert(0, "/opt/trn_rl_repo")
from contextlib import ExitStack
import numpy as np
import concourse.bass as bass
import concourse.mybir as mybir
from concourse import bacc
from concourse.tile import TileContext
from concourse.bass_utils import run_bass_kernel_spmd

F32 = mybir.dt.float32
BF16 = mybir.dt.bfloat16
AF = mybir.ActivationFunctionType
OP = mybir.AluOpType
AX = mybir.AxisListType

D, NB, H = 1024, 512, 4096
NCORE = 8
T = 1024          # tokens per core
P = 128
TI = T // P       # 8 token tiles
DK = D // P       # 8 contraction tiles over D
HJ = H // P       # 32 tiles over ffn hidden
NBJ = NB // P     # 4 tiles over basis dim
TAU = 10.0
EPS = 1e-6
GELU = (AF.Identity if os.environ.get("KERNEL_NO_GELU") else AF.Gelu)


def _build():
    nc = bacc.Bacc("TRN2", target_bir_lowering=False, debug=False, num_devices=NCORE)
    xt_e = nc.declare_dram_parameter("xt", [D, T], F32, isOutput=False)
    w1t_e = nc.declare_dram_parameter("w1t", [D, H], F32, isOutput=False)
    w2t_e = nc.declare_dram_parameter("w2t", [H, D], F32, isOutput=False)
    rwt_e = nc.declare_dram_parameter("rwt", [D, NB], F32, isOutput=False)
    ut_e = nc.declare_dram_parameter("ut", [D, NB], F32, isOutput=False)
    v_e = nc.declare_dram_parameter("v", [NB, D], F32, isOutput=False)
    gcol_e = nc.declare_dram_parameter("gcol", [P, DK], F32, isOutput=False)
    bcol_e = nc.declare_dram_parameter("bcol", [P, DK], F32, isOutput=False)
    rb_e = nc.declare_dram_parameter("rb", [1, NB], F32, isOutput=False)
    gam_e = nc.declare_dram_parameter("gam", [1, D], F32, isOutput=False)
    eye_e = nc.declare_dram_parameter("eye", [P, P], F32, isOutput=False)
    out_e = nc.declare_dram_parameter("out", [T, D], F32, isOutput=True)

    xt_v = xt_e[:].rearrange("(ko p) t -> p ko t", p=P)      # [128, DK, T]
    w1t_v = w1t_e[:].rearrange("(ko p) h -> p ko h", p=P)    # [128, DK, H]
    w2t_v = w2t_e[:].rearrange("(ho p) d -> p ho d", p=P)    # [128, HJ, D]
    rwt_v = rwt_e[:].rearrange("(ko p) n -> p ko n", p=P)    # [128, DK, NB]
    ut_v = ut_e[:].rearrange("(ko p) n -> p ko n", p=P)      # [128, DK, NB]
    v_v = v_e[:].rearrange("(no p) d -> p no d", p=P)        # [128, NBJ, D]
    out_v = out_e[:].rearrange("(to p) d -> p to d", p=P)    # [128, TI, D]

    with TileContext(nc) as tc, ExitStack() as ctx:
        const = ctx.enter_context(tc.tile_pool(name="const", bufs=1))
        persist = ctx.enter_context(tc.tile_pool(name="persist", bufs=1))

        ones_row = const.tile([1, P], BF16)
        nc.vector.memset(ones_row[:], 1.0)
        ones_bc = const.tile([P, P], BF16)
        nc.vector.memset(ones_bc[:], 1.0)
        epsb = const.tile([P, 1], F32)
        nc.vector.memset(epsb[:], 1e-5)
        gcol = const.tile([P, DK], F32)
        bcol = const.tile([P, DK], F32)
        nc.sync.dma_start(gcol[:], gcol_e[:])
        nc.sync.dma_start(bcol[:], bcol_e[:])
        ident = const.tile([P, P], BF16)

        xtb = persist.tile([P, DK, T], BF16)      # 16K/part
        vs = persist.tile([P, NBJ, D], BF16)      # 8K
        gt = persist.tile([P, NBJ, T], BF16)      # 8K
        gall = persist.tile([P, TI, NB], BF16)    # 8K  (G, pre-transpose)
        zsall = persist.tile([P, TI, NB], BF16)   # 8K  (masked alpha)
        hfall = persist.tile([P, TI, NB], BF16)   # 8K  (h_full)
        rs_t = persist.tile([P, TI], F32)
        mrs_t = persist.tile([P, TI], F32)
        sall = persist.tile([P, TI], F32)
        thall = persist.tile([P, TI], F32)

        with tc.tile_pool(name="pares", bufs=1) as pares:
            wg = pares.tile([P, DK, NB], BF16)
            un = pares.tile([P, DK, NB], BF16)
            sg_b = pares.tile([P, NB], F32)
            c_b = pares.tile([P, NB], F32)

            pp0_ctx = ExitStack()
            with tc.tile_pool(name="p0", bufs=1) as p0, \
                 tc.tile_pool(name="p0s", bufs=2) as p0s, \
                 tc.tile_pool(name="p0b", bufs=2) as p0b, \
                 tc.tile_pool(name="pa", bufs=3) as pa, \
                 tc.tile_pool(name="pa_sm", bufs=2) as pa_sm, \
                 tc.tile_pool(name="ppa", bufs=6, space="PSUM") as ppa:
                # ---- bulk DMAs, ordered by need: x first, tables behind ----
                pp0 = pp0_ctx.enter_context(
                    tc.tile_pool(name="pp0", bufs=2, space="PSUM"))
                eyef = p0.tile([P, P], F32, tag="eyef")
                nc.sync.dma_start(eyef[:], eye_e[:])
                nc.gpsimd.tensor_copy(ident[:], eyef[:])
                rwtf = p0s.tile([P, DK, NB], F32, tag="stageB")
                nc.sync.dma_start(rwtf[:], rwt_v[:])
                rb_f = p0.tile([1, NB], F32, tag="rb_f")
                gam_f = p0.tile([1, D], F32, tag="gam_f")
                nc.sync.dma_start(rb_f[:], rb_e[:])
                nc.sync.dma_start(gam_f[:], gam_e[:])
                rb_row = p0.tile([1, NB], BF16, tag="rb_row")
                gam_row = p0.tile([1, D], BF16, tag="gam_row")
                nc.gpsimd.tensor_copy(rb_row[:], rb_f[:])
                nc.gpsimd.tensor_copy(gam_row[:], gam_f[:])

                utf = p0s.tile([P, DK, NB], F32, tag="stageB")
                nc.sync.dma_start(utf[:], ut_v[:])
                # DVE: router table casts first (rwt lands first), then x
                rwb = p0.tile([P, DK, NB], BF16, tag="rwb")
                for dk in range(DK):
                    nc.vector.tensor_copy(rwb[:, dk, :], rwtf[:, dk, :])
                for dk in range(DK):
                    xtf = p0b.tile([P, T], F32, tag="xtf")
                    nc.sync.dma_start(xtf[:], xt_v[:, dk, :])
                    nc.vector.tensor_copy(xtb[:, dk, :], xtf[:])
                for dk in range(DK):
                    nc.vector.tensor_scalar(wg[:, dk, :], rwtf[:, dk, :],
                                            gcol[:, dk : dk + 1], None, OP.mult)

                def emit_prep_mms():
                    gam_b = p0.tile([P, D], F32, tag="gam_b")
                    for half in range(2):
                        gam_ps = pp0.tile([P, 512], F32, tag="ps512")
                        nc.tensor.matmul(gam_ps[:], ones_row[:],
                                         gam_row[:, half * 512 : (half + 1) * 512],
                                         start=True, stop=True)
                        nc.vector.tensor_copy(
                            gam_b[:, half * 512 : (half + 1) * 512], gam_ps[:])
                    gbc = p0.tile([P, DK, P], BF16, tag="gbc")
                    bbc = p0.tile([P, DK, P], BF16, tag="bbc")
                    for dk in range(DK):
                        nc.gpsimd.tensor_copy(
                            gbc[:, dk, :], gcol[:, dk : dk + 1].to_broadcast([P, P]))
                        nc.gpsimd.tensor_copy(
                            bbc[:, dk, :], bcol[:, dk : dk + 1].to_broadcast([P, P]))
                    sg_ps = pp0.tile([P, NB], F32, tag="ps512")
                    for dk in range(DK):
                        nc.tensor.matmul(sg_ps[:], gbc[:, dk, :], rwb[:, dk, :],
                                         start=(dk == 0), stop=(dk == DK - 1))
                    nc.vector.tensor_copy(sg_b[:], sg_ps[:])
                    c_ps = pp0.tile([P, NB], F32, tag="ps512")
                    for dk in range(DK):
                        nc.tensor.matmul(c_ps[:], bbc[:, dk, :], rwb[:, dk, :],
                                         start=(dk == 0), stop=False)
                    nc.tensor.matmul(c_ps[:], ones_row[:], rb_row[:],
                                     start=False, stop=True)
                    nc.vector.tensor_copy(c_b[:], c_ps[:])
                    # U norms
                    nsq_ps = pp0.tile([P, NB], F32, tag="ps512")
                    for dk in range(DK):
                        usq = p0b.tile([P, NB], BF16, tag="usq")
                        useng = nc.vector if dk % 2 == 0 else nc.gpsimd
                        useng.tensor_tensor(usq[:], utf[:, dk, :],
                                            utf[:, dk, :], OP.mult)
                        nc.tensor.matmul(nsq_ps[:], ones_bc[:], usq[:],
                                         start=(dk == 0), stop=(dk == DK - 1))
                    rno = p0b.tile([P, NB], F32, tag="rno")
                    nc.scalar.activation(rno[:], nsq_ps[:], AF.Ln)
                    nc.scalar.activation(rno[:], rno[:], AF.Exp, scale=-0.5)
                    nc.vector.tensor_scalar_min(rno[:], rno[:], 1.0 / EPS)
                    for dk in range(DK):
                        ueng = nc.vector if dk % 2 == 0 else nc.gpsimd
                        ueng.tensor_tensor(un[:, dk, :], utf[:, dk, :],
                                           rno[:], OP.mult)
                    return gam_b

                gam_b = emit_prep_mms()
                # LN stats via ones-matmuls on x (PE) + transposes; no xn input
                sum_b = p0.tile([P, T], F32, tag="sum_b")
                sq_b = p0.tile([P, T], F32, tag="sq_b")
                for half in range(2):
                    hsl = slice(half * 512, (half + 1) * 512)
                    sps = pp0.tile([P, 512], F32, tag="ps512")
                    for dk in range(DK):
                        nc.tensor.matmul(sps[:], ones_bc[:], xtb[:, dk, hsl],
                                         start=(dk == 0), stop=(dk == DK - 1))
                    nc.vector.tensor_copy(sum_b[:, hsl], sps[:])
                for half in range(2):
                    hsl = slice(half * 512, (half + 1) * 512)
                    sps = pp0.tile([P, 512], F32, tag="ps512")
                    for dk in range(DK):
                        xsq = p0b.tile([P, 512], BF16, tag="xsq")
                        nc.vector.tensor_tensor(xsq[:], xtb[:, dk, hsl],
                                                xtb[:, dk, hsl], OP.mult)
                        nc.tensor.matmul(sps[:], ones_bc[:], xsq[:],
                                         start=(dk == 0), stop=(dk == DK - 1))
                    nc.vector.tensor_copy(sq_b[:, hsl], sps[:])
                mu_c = p0b.tile([P, TI], F32, tag="mu_c")
                sq_c = p0b.tile([P, TI], F32, tag="sq_c")
                for ti in range(TI):
                    tsl = slice(ti * P, (ti + 1) * P)
                    pts = pp0.tile([P, P], F32, tag="ps512")
                    nc.tensor.transpose(pts[:], sum_b[:, tsl], eyef[:])
                    nc.vector.tensor_copy(mu_c[:, ti : ti + 1], pts[:, 0:1])
                    ptq = pp0.tile([P, P], F32, tag="ps512")
                    nc.tensor.transpose(ptq[:], sq_b[:, tsl], eyef[:])
                    nc.vector.tensor_copy(sq_c[:, ti : ti + 1], ptq[:, 0:1])
                mu_all = p0b.tile([P, TI], F32, tag="mu_all")
                var_all = p0b.tile([P, TI], F32, tag="var_all")
                nc.vector.tensor_scalar_mul(mu_all[:], mu_c[:], 1.0 / D)
                nc.vector.tensor_scalar_mul(sq_c[:], sq_c[:], 1.0 / D)
                nc.vector.tensor_tensor(var_all[:], mu_all[:], mu_all[:], OP.mult)
                nc.vector.tensor_sub(var_all[:], sq_c[:], var_all[:])
                lnv = p0b.tile([P, TI], F32, tag="lnv")
                nc.scalar.activation(lnv[:], var_all[:], AF.Ln, bias=epsb[:])
                nc.scalar.activation(rs_t[:], lnv[:], AF.Exp, scale=-0.5)
                nc.vector.scalar_tensor_tensor(mrs_t[:], mu_all[:], -1.0,
                                               rs_t[:], OP.mult, OP.mult)

                # ---- A pass 1a: router matmuls + LN fixup ----
                rf_l, e_l, al_l = [], [], []

                for ti in range(TI):
                    tsl = slice(ti * P, (ti + 1) * P)
                    r0 = ppa.tile([P, NB], F32, tag="pA")
                    for dk in range(DK):
                        nc.tensor.matmul(r0[:], xtb[:, dk, tsl], wg[:, dk, :],
                                         start=(dk == 0), stop=(dk == DK - 1))
                    rf = pa.tile([P, NB], F32, tag="rf")
                    nc.vector.scalar_tensor_tensor(
                        rf[:], r0[:], rs_t[:, ti : ti + 1], c_b[:],
                        OP.mult, OP.add)
                    nc.vector.scalar_tensor_tensor(
                        rf[:], sg_b[:], mrs_t[:, ti : ti + 1], rf[:],
                        OP.mult, OP.add)
                    nc.gpsimd.tensor_scalar(rf[:], rf[:], TAU, -TAU,
                                            OP.min, OP.max)
                    rf_l.append(rf)

                # ---- A pass 1b: h_full matmuls (evict via ACT to SBUF) ----
                for ti in range(TI):
                    tsl = slice(ti * P, (ti + 1) * P)
                    hf = ppa.tile([P, NB], F32, tag="pA")
                    for dk in range(DK):
                        nc.tensor.matmul(hf[:], xtb[:, dk, tsl], un[:, dk, :],
                                         start=(dk == 0), stop=(dk == DK - 1))
                    nc.vector.tensor_copy(hfall[:, ti, :], hf[:])

                pp0_ctx.close()

                # ---- V norms (DVE; vf DMA behind tables on sync queue) ----
                vf = p0.tile([P, NBJ, D], F32, tag="stageA")
                nc.sync.dma_start(vf[:], v_v[:])
                vss = p0b.tile([P, NBJ], F32, tag="vss")
                rnv = p0b.tile([P, NBJ], F32, tag="rnv")
                for nbj in range(NBJ):
                    vsq = p0b.tile([P, D], F32, tag="vsq")
                    nc.gpsimd.tensor_tensor(vsq[:], vf[:, nbj, :], vf[:, nbj, :],
                                            OP.mult)
                    nc.vector.reduce_sum(vss[:, nbj : nbj + 1], vsq[:], axis=AX.X)
                nc.scalar.activation(rnv[:], vss[:], AF.Ln)
                nc.scalar.activation(rnv[:], rnv[:], AF.Exp, scale=-0.5)
                nc.vector.tensor_scalar_min(rnv[:], rnv[:], 1.0 / EPS)
                for nbj in range(NBJ):
                    nc.vector.scalar_tensor_tensor(
                        vs[:, nbj, :], vf[:, nbj, :], rnv[:, nbj : nbj + 1],
                        gam_b[:], OP.mult, OP.mult)

                # ---- A passes 2-6: softplus, top-8, q, G ----
                for ti in range(TI):
                    e_sb = pa.tile([P, NB], F32, tag="e_sb")
                    nc.scalar.activation(e_sb[:], rf_l[ti][:], AF.Exp)
                    e_l.append(e_sb)
                for ti in range(TI):
                    alpha = pa.tile([P, NB], F32, tag="alpha")
                    nc.scalar.activation(alpha[:], e_l[ti][:], AF.Ln, bias=1.0)
                    al_l.append(alpha)
                for ti in range(TI):
                    alpha = al_l[ti]
                    m8 = pa_sm.tile([P, 8], F32, tag="m8")
                    nc.vector.max(out=m8[:], in_=alpha[:])
                    nc.vector.reduce_sum(sall[:, ti : ti + 1], m8[:], axis=AX.X)
                    repl = pa.tile([P, NB], F32, tag="repl")
                    nc.vector.match_replace(out=repl[:], in_to_replace=m8[:],
                                            in_values=alpha[:], imm_value=0.0)
                    nc.vector.tensor_sub(zsall[:, ti, :], alpha[:], repl[:])
                for ti in range(TI):
                    nc.scalar.activation(thall[:, ti : ti + 1],
                                         sall[:, ti : ti + 1], AF.Tanh)
                for ti in range(TI):
                    sp = pa_sm.tile([P, 1], F32, tag="sp")
                    nc.vector.tensor_scalar_add(sp[:], sall[:, ti : ti + 1], EPS)
                    nc.vector.reciprocal(sp[:], sp[:])
                    q = pa_sm.tile([P, 1], F32, tag="q")
                    nc.vector.tensor_tensor(q[:], thall[:, ti : ti + 1], sp[:],
                                            OP.mult)
                    nc.vector.scalar_tensor_tensor(
                        gall[:, ti, :], zsall[:, ti, :], q[:], hfall[:, ti, :],
                        OP.mult, OP.mult)

        # ============ B/C: FFN + output, token-halved ============
        with tc.tile_pool(name="bigw", bufs=2) as bigw, \
             tc.tile_pool(name="pw2", bufs=3) as pw2, \
             tc.tile_pool(name="bigp", bufs=1) as bigp, \
             tc.tile_pool(name="pb", bufs=6) as pb, \
             tc.tile_pool(name="ppt", bufs=3, space="PSUM") as ppt:

            def ffn1_half(half, ppb, emit_t=None):
                hsl = slice(half * 512, (half + 1) * 512)
                gh = bigp.tile([P, HJ, 512], BF16, tag="gh")
                for hj in range(HJ):
                    if emit_t is not None and 16 <= hj < 24:
                        emit_t(hj - 16)
                    w1f = pb.tile([P, DK, P], F32, tag="w1f")
                    nc.sync.dma_start(w1f[:], w1t_v[:, :, hj * P : (hj + 1) * P])
                    w1c = pb.tile([P, DK, P], BF16, tag="w1c")
                    if hj % 2 == 0:
                        nc.scalar.copy(
                            w1c[:].rearrange("p a b -> p (a b)"),
                            w1f[:].rearrange("p a b -> p (a b)"))
                    else:
                        nc.gpsimd.tensor_copy(
                            w1c[:].rearrange("p a b -> p (a b)"),
                            w1f[:].rearrange("p a b -> p (a b)"))
                    hps = ppb.tile([P, 512], F32, tag="hps")
                    for dk in range(DK):
                        nc.tensor.matmul(hps[:], w1c[:, dk, :], xtb[:, dk, hsl],
                                         start=(dk == 0), stop=(dk == DK - 1))
                    nc.scalar.activation(gh[:, hj, :], hps[:], GELU)
                return gh

            def out_half(half, gh, pc, ppc):
                for dh in range(2):
                    dsl = slice(dh * 512, (dh + 1) * 512)
                    w2h = bigw.tile([P, HJ, 512], BF16, tag="w2h")
                    for ch in range(HJ // 2):
                        w2f = pw2.tile([P, 2, 512], F32, tag="w2f")
                        nc.sync.dma_start(
                            w2f[:], w2t_v[:, ch * 2 : (ch + 1) * 2, dsl])
                        nc.vector.tensor_copy(
                            w2h[:, ch * 2 : (ch + 1) * 2, :].rearrange(
                                "p a b -> p (a b)"),
                            w2f[:].rearrange("p a b -> p (a b)"))
                    for ti4 in range(4):
                        ti = half * 4 + ti4
                        tsl = slice(ti * P, (ti + 1) * P)
                        t4sl = slice(ti4 * P, (ti4 + 1) * P)
                        ops = ppc.tile([P, 512], F32, tag="ops")
                        for hj in range(HJ):
                            nc.tensor.matmul(ops[:], gh[:, hj, t4sl],
                                             w2h[:, hj, :],
                                             start=(hj == 0), stop=False)
                        for nbj in range(NBJ):
                            nc.tensor.matmul(ops[:], gt[:, nbj, tsl],
                                             vs[:, nbj, dsl],
                                             start=False, stop=(nbj == NBJ - 1))
                        o_sb = pc.tile([P, 512], F32, tag="o_sb")
                        nc.vector.tensor_copy(o_sb[:], ops[:])
                        nc.sync.dma_start(out_v[:, ti, dsl], o_sb[:])

            def emit_transpose(ti):
                tsl = slice(ti * P, (ti + 1) * P)
                for nbj in range(NBJ):
                    pt = ppt.tile([P, P], BF16, tag="pt")
                    nc.tensor.transpose(
                        pt[:], gall[:, ti, nbj * P : (nbj + 1) * P], ident[:])
                    nc.vector.tensor_copy(gt[:, nbj, tsl], pt[:])

            with tc.tile_pool(name="ppb0", bufs=3, space="PSUM") as ppb0:
                gh0 = ffn1_half(0, ppb0, emit_t=emit_transpose)

            with tc.tile_pool(name="pc", bufs=2) as pc, \
                 tc.tile_pool(name="ppc", bufs=3, space="PSUM") as ppc, \
                 tc.tile_pool(name="ppb1", bufs=2, space="PSUM") as ppb1:
                out_half(0, gh0, pc, ppc)
                gh1 = ffn1_half(1, ppb1)
                out_half(1, gh1, pc, ppc)

    nc.compile()
    return nc


_cached_nc = None
_EYE = np.eye(P, dtype=np.float32)


def kernel(x, W1, W2, ln_g, ln_b, router_W, router_b, raw_U, raw_V, gamma):
    global _cached_nc
    x = np.ascontiguousarray(np.asarray(x, np.float32)).reshape(-1, D)
    w1t = np.ascontiguousarray(np.asarray(W1, np.float32).T)
    w2t = np.ascontiguousarray(np.asarray(W2, np.float32).T)
    rwt = np.ascontiguousarray(np.asarray(router_W, np.float32).T)
    utt = np.ascontiguousarray(np.asarray(raw_U, np.float32).T)
    vv = np.ascontiguousarray(np.asarray(raw_V, np.float32))
    gcol = np.ascontiguousarray(np.asarray(ln_g, np.float32).reshape(DK, P).T)
    bcol = np.ascontiguousarray(np.asarray(ln_b, np.float32).reshape(DK, P).T)
    rb = np.ascontiguousarray(np.asarray(router_b, np.float32).reshape(1, NB))
    gam = np.ascontiguousarray(np.asarray(gamma, np.float32).reshape(1, D))

    if _cached_nc is None:
        _cached_nc = _build()
    nc = _cached_nc

    in_maps = []
    for c in range(NCORE):
        shard = x[c * T : (c + 1) * T]
        in_maps.append({
            "xt": np.ascontiguousarray(shard.T),
            "w1t": w1t, "w2t": w2t, "rwt": rwt, "ut": utt, "v": vv,
            "gcol": gcol, "bcol": bcol, "rb": rb, "gam": gam,
            "eye": _EYE,
        })
    res = run_bass_kernel_spmd(nc, in_maps, list(range(NCORE)))
    kernel._last_results = res
    out = np.concatenate([res.results[c]["out"] for c in range(NCORE)], axis=0)
    return out.reshape(4, 2048, D)

